# revision 15
# baseline (speedup 1.0000x reference)
"""Trainium2 Bass kernel for the block-diagonal equivariant linear
(irreps 256x0e + 256x1o + 128x2e, B=32768, D=1664) on 8 NeuronCores.

Strategy: data-parallel over batch (4096 rows/core), weights replicated.
All HBM traffic is bf16 and all data-layout work is hoisted to the host,
so the device runs a pure block-diagonal GEMM at the bf16 DMA roofline
(13.6MB in + 13.6MB out per core):

- host pre-casts x to bf16 and pre-arranges it chunk-major: per batch
  chunk, SBUF partition u holds 13 planes x n batch values contiguously
  -> each load is one flat 2D DMA with multi-KB contiguous runs;
- matmuls run weights-stationary / x-moving: out block [128 w, n b] per
  (segment, component, w-chunk), accumulated over u-chunks in one PSUM
  bank; bias is a rank-1 PE matmul (bias x ones) on the scalar blocks;
- output goes back in block-major [128 w, block*n+b] bf16 layout; the
  host de-interleaves features/batch and upcasts to fp32 per chunk.

Pipeline shaping: weights (shrunk to [128,1152] + a [1,768] bias/ones
row) lead the Sync HWDGE ring, then all loads follow on it; stores ride
the Scalar HWDGE ring and each chunk is stored in two halves (blocks
0-6 / 7-12, separate SBUF tiles) so the store stream starts as early as
possible; the first chunks are small (128/384 rows) to shorten the
pipeline ramp.  PSUM->SBUF casts alternate Vector/Scalar.
"""

import math
import sys

if "/opt/trn_rl_repo" not in sys.path:
    sys.path.insert(0, "/opt/trn_rl_repo")

import ml_dtypes
import numpy as np

import concourse.tile as tile
from concourse import bacc, mybir
from concourse.bass_utils import run_bass_kernel_spmd

# Problem constants (hardcoded; see module docstring).
DIM = 1664
B_TOTAL = 32768
N_CORES = 8
B_CORE = B_TOTAL // N_CORES  # 4096
CHUNK_B = 512
CH_SIZES = [128, 384] + [512] * 7
assert sum(CH_SIZES) == B_CORE
CH_STARTS = [sum(CH_SIZES[:i]) for i in range(len(CH_SIZES))]
N_CHUNKS = len(CH_SIZES)

# (feature_offset, mul, ir_dim) per segment of the flat feature vector.
SEGS = [(0, 256, 1), (256, 256, 3), (1024, 128, 5)]

# Weight packing columns: per segment, per u-chunk, a [128, mul] block.
_wcols = {}
_wcol = 0
for _si, (s, mul, d) in enumerate(SEGS):
    _wcols[_si] = []
    for c in range(mul // 128):
        _wcols[_si].append(_wcol)
        _wcol += mul
WPK_COLS = _wcol  # 1152
# row tensor (row 0 of wt cols [WPK_COLS:]): bias (256) then ones (512)
BIAS_COL = WPK_COLS
ONES_COL = WPK_COLS + 256
WROW = 256 + CHUNK_B  # 768
WTOT = WPK_COLS + WROW  # 1920

# Input planes: plane (si, i, uc) -> index, in si -> i -> uc order.
PLANE_IDX = {}
PLANE_FEAT = []
for _si, (s, mul, d) in enumerate(SEGS):
    for _i in range(d):
        for _uc in range(mul // 128):
            PLANE_IDX[(_si, _i, _uc)] = len(PLANE_FEAT)
            PLANE_FEAT.append((s + _i + d * 128 * _uc, d))
N_PLANES = len(PLANE_FEAT)  # 13
assert N_PLANES == 13

# Output blocks: block g -> (si, i, wc); out[wc*128+p (partition), b].
# Emission (and copy-completion) order is g0..g12; the chunk is stored
# in two halves: A = blocks 0-6, B = blocks 7-12.
BLOCKS = []
for _wc in range(2):
    for _i in range(3):
        BLOCKS.append((1, _i, _wc))
BLOCKS += [(0, 0, 0), (0, 0, 1)]
BLOCKS += [(2, _i, 0) for _i in range(5)]
G_IDX = {blk: g for g, blk in enumerate(BLOCKS)}
N_BLOCKS = len(BLOCKS)  # 13
N_A = 7  # blocks in store-half A
N_B = N_BLOCKS - N_A  # 6

TOT_COLS = N_PLANES * B_CORE  # flat per-partition DRAM cols (in & out)

# Host-side index tables.
FEAT_OF = np.empty((128, N_PLANES), dtype=np.intp)
for _pl, (_off, _step) in enumerate(PLANE_FEAT):
    FEAT_OF[:, _pl] = _off + _step * np.arange(128)
INV_COL = np.empty(DIM, dtype=np.intp)
for _g, (_si, _i, _wc) in enumerate(BLOCKS):
    _s, _mul, _d = SEGS[_si]
    _w = _wc * 128 + np.arange(128)
    INV_COL[_s + _d * _w + _i] = _g * 128 + np.arange(128)


def _host_weights(ws: np.ndarray, bs: np.ndarray):
    """Pack per-segment weights (1/sqrt(mul) folded in) as [128, 1152]
    bf16, plus a [1, 768] bias+ones row."""
    wpk = np.zeros((128, WPK_COLS), dtype=np.float32)
    off = 0
    for si, (s, mul, d) in enumerate(SEGS):
        w = ws[off : off + mul * mul].reshape(mul, mul) * np.float32(
            1.0 / math.sqrt(mul)
        )
        off += mul * mul
        for c, col in enumerate(_wcols[si]):
            wpk[:, col : col + mul] = w[c * 128 : (c + 1) * 128, :]
    wrow = np.empty((1, WROW), dtype=np.float32)
    wrow[0, :256] = bs
    wrow[0, 256:] = 1.0
    return wpk.astype(ml_dtypes.bfloat16), wrow.astype(ml_dtypes.bfloat16)


def _host_planes(x: np.ndarray) -> np.ndarray:
    """x [B_TOTAL, DIM] fp32 -> xh [N_CORES, 128, TOT_COLS] bf16:
    per chunk, partition u holds plane-major contiguous batch runs."""
    X = x.reshape(N_CORES, B_CORE, DIM)
    xh = np.empty((N_CORES, 128, TOT_COLS), dtype=ml_dtypes.bfloat16)
    for b0, n in zip(CH_STARTS, CH_SIZES):
        cb = N_PLANES * b0
        blk = X[:, b0 : b0 + n, :][:, :, FEAT_OF]  # [C, n, u, pl]
        xh[:, :, cb : cb + N_PLANES * n] = (
            blk.transpose(0, 2, 3, 1).reshape(N_CORES, 128, N_PLANES * n)
        )
    return xh


def _host_out(outs) -> np.ndarray:
    """Device outputs [128, TOT_COLS] bf16 per core -> [B_TOTAL, DIM] fp32."""
    dev = np.stack([np.asarray(o) for o in outs])  # [C, p, cols]
    out = np.empty((N_CORES, B_CORE, DIM), dtype=np.float32)
    for b0, n in zip(CH_STARTS, CH_SIZES):
        cb = N_BLOCKS * b0
        blk = dev[:, :, cb : cb + N_BLOCKS * n].reshape(
            N_CORES, 128, N_BLOCKS, n
        )
        arr = blk.transpose(0, 3, 2, 1).reshape(N_CORES, n, N_BLOCKS * 128)
        out[:, b0 : b0 + n, :] = arr[:, :, INV_COL]
    return out.reshape(B_TOTAL, DIM)


def build_program(x_bufs: int = 5, out_bufs: int = 6, ps_bufs: int = 4):
    """Build + compile the per-core SPMD program. Returns compiled nc."""
    f32 = mybir.dt.float32
    bf16 = mybir.dt.bfloat16

    nc = bacc.Bacc("TRN2", target_bir_lowering=False, debug=False)
    xh_ap = nc.dram_tensor(
        "xh", [128, TOT_COLS], bf16, kind="ExternalInput"
    ).ap()
    wpk_ap = nc.dram_tensor(
        "wpk", [128, WPK_COLS], bf16, kind="ExternalInput"
    ).ap()
    wrow_ap = nc.dram_tensor("wrow", [1, WROW], bf16, kind="ExternalInput").ap()
    out_ap = nc.dram_tensor(
        "out", [128, TOT_COLS], bf16, kind="ExternalOutput"
    ).ap()

    with tile.TileContext(nc) as tc:
        with (
            tc.tile_pool(name="consts", bufs=1) as cpool,
            tc.tile_pool(name="x", bufs=x_bufs) as x_pool,
            tc.tile_pool(name="outsA", bufs=out_bufs) as outA_pool,
            tc.tile_pool(name="outsB", bufs=out_bufs) as outB_pool,
            tc.tile_pool(name="psO", bufs=ps_bufs, space="PSUM") as psO_pool,
        ):
            # weights lead the Sync ring so they land before load 0 ends
            wt = cpool.tile([128, WTOT], bf16)
            nc.sync.dma_start(wt[:, :WPK_COLS], wpk_ap[:])
            nc.sync.dma_start(wt[0:1, WPK_COLS:], wrow_ap[:])

            x_tiles = {}
            out_tiles = {}

            def load_chunk(ch):
                b0, n = CH_STARTS[ch], CH_SIZES[ch]
                cb = N_PLANES * b0
                xt = x_pool.tile([128, N_PLANES * CHUNK_B], bf16, tag="x")
                nc.sync.dma_start(
                    xt[:, : N_PLANES * n], xh_ap[:, cb : cb + N_PLANES * n]
                )
                x_tiles[ch] = xt
                ova = outA_pool.tile(
                    [128, N_A * CHUNK_B], bf16, tag="outA", name=f"outa{ch}"
                )
                ovb = outB_pool.tile(
                    [128, N_B * CHUNK_B], bf16, tag="outB", name=f"outb{ch}"
                )
                out_tiles[ch] = (ova, ovb)

            def store_half(ch, half):
                b0, n = CH_STARTS[ch], CH_SIZES[ch]
                cb = N_BLOCKS * b0
                ova, ovb = out_tiles[ch]
                # tail-chunk stores ride the Sync ring, which is idle once
                # all loads are issued; earlier stores use the Scalar ring
                eng = nc.sync if ch >= N_CHUNKS - 3 else nc.scalar
                if half == 0:
                    eng.dma_start(
                        out_ap[:, cb : cb + N_A * n], ova[:, : N_A * n]
                    )
                else:
                    eng.dma_start(
                        out_ap[:, cb + N_A * n : cb + N_BLOCKS * n],
                        ovb[:, : N_B * n],
                    )
                    del out_tiles[ch]

            # PSUM blocks are paired into [128, 2*CHUNK_B] tiles so each
            # PSUM->SBUF cast moves two blocks per instruction (amortizes
            # the fixed per-instruction engine cost).  Pairs never cross
            # the A/B store-half boundary.
            PAIRS = [(0, 1), (2, 3), (4, 5), (6,), (7, 8), (9, 10), (11, 12)]
            PAIR_OF = {}
            for _pi, _pr in enumerate(PAIRS):
                for _j, _g in enumerate(_pr):
                    PAIR_OF[_g] = (_pi, _j)

            copy_flip = [0]

            def chunk_phase(ch):
                n = CH_SIZES[ch]
                xt = x_tiles.pop(ch)
                ova, ovb = out_tiles[ch]
                pairtiles = {}
                pair_done = {}

                def pslice(g):
                    # pair partners sit at CHUNK_B-aligned offsets so a
                    # matmul output region never crosses a PSUM bank
                    pi, j = PAIR_OF[g]
                    if pi not in pairtiles:
                        pairtiles[pi] = psO_pool.tile(
                            [128, 2 * CHUNK_B], f32, tag="psO",
                            name=f"ps{ch}_{pi}",
                        )
                        pair_done[pi] = 0
                    t = pairtiles[pi]
                    return t[:, j * CHUNK_B : j * CHUNK_B + n]

                def ovdst(g, w):
                    if g < N_A:
                        return ova[:, g * n : g * n + w]
                    return ovb[:, (g - N_A) * n : (g - N_A) * n + w]

                def emit1(dst, src):
                    if copy_flip[0] % 2 == 0:
                        nc.vector.tensor_copy(dst, src)
                    else:
                        nc.scalar.copy(dst, src)
                    copy_flip[0] += 1

                def finish(g):
                    """Mark block g complete; copy its pair out when full.
                    Full pairs are copied as two halves issued to BOTH copy
                    engines at once -- the PSUM pair frees ~2x sooner, which
                    is what paces the PE."""
                    pi, _ = PAIR_OF[g]
                    pair_done[pi] += 1
                    if pair_done[pi] < len(PAIRS[pi]):
                        return
                    t = pairtiles[pi]
                    gs = PAIRS[pi]
                    if len(gs) == 1:
                        emit1(ovdst(gs[0], n), t[:, :n])
                    else:
                        for j, gj in enumerate(gs):
                            emit1(
                                ovdst(gj, n),
                                t[:, j * CHUNK_B : j * CHUNK_B + n],
                            )

                def xpl(si, i, uc):
                    pl = PLANE_IDX[(si, i, uc)]
                    return xt[:, pl * n : pl * n + n]

                def wblk(si, uc, wc):
                    c0 = _wcols[si][uc] + wc * 128
                    return wt[:, c0 : c0 + 128]

                # seg1: 3 components x 2 w-chunks, accumulate over u-chunks;
                # i-inner so each stationary weight block feeds 3 matmuls
                for wc in range(2):
                    for uc in range(2):
                        for i in range(3):
                            nc.tensor.matmul(
                                pslice(G_IDX[(1, i, wc)]),
                                wblk(1, uc, wc),
                                xpl(1, i, uc),
                                start=(uc == 0),
                                stop=(uc == 1),
                            )
                    for i in range(3):
                        finish(G_IDX[(1, i, wc)])
                # seg0: 2 w-chunks, accumulate over u-chunks + rank-1 bias
                for wc in range(2):
                    g = G_IDX[(0, 0, wc)]
                    for uc in range(2):
                        nc.tensor.matmul(
                            pslice(g),
                            wblk(0, uc, wc),
                            xpl(0, 0, uc),
                            start=(uc == 0),
                            stop=False,
                        )
                    nc.tensor.matmul(
                        pslice(g),
                        wt[0:1, BIAS_COL + wc * 128 : BIAS_COL + wc * 128 + 128],
                        wt[0:1, ONES_COL : ONES_COL + n],
                        start=False,
                        stop=True,
                    )
                    finish(g)
                    if wc == 0:
                        store_half(ch, 0)  # blocks 0-6 complete
                # seg2: 5 components, single u-chunk (shared stationary)
                for i in range(5):
                    g = G_IDX[(2, i, 0)]
                    nc.tensor.matmul(
                        pslice(g), wblk(2, 0, 0), xpl(2, i, 0),
                        start=True, stop=True,
                    )
                    finish(g)
                store_half(ch, 1)

            for ch in range(min(x_bufs, N_CHUNKS)):
                load_chunk(ch)
            loaded = min(x_bufs, N_CHUNKS)
            for ch in range(N_CHUNKS):
                if loaded < N_CHUNKS:
                    load_chunk(loaded)
                    loaded += 1
                chunk_phase(ch)

    nc.compile()
    return nc


_CACHE: dict = {}


def prep_in_maps(ws: np.ndarray, bs: np.ndarray, x: np.ndarray):
    wpk, wrow = _host_weights(
        np.asarray(ws, dtype=np.float32), np.asarray(bs, dtype=np.float32)
    )
    xh = _host_planes(np.asarray(x, dtype=np.float32))
    return [{"xh": xh[i], "wpk": wpk, "wrow": wrow} for i in range(N_CORES)]


def kernel(ws: np.ndarray, bs: np.ndarray, x: np.ndarray) -> np.ndarray:
    if "nc" not in _CACHE:
        _CACHE["nc"] = build_program()
    nc = _CACHE["nc"]
    in_maps = prep_in_maps(ws, bs, x)
    res = run_bass_kernel_spmd(nc, in_maps, list(range(N_CORES)))
    return _host_out([r["out"] for r in res.results])


# revision 16
# speedup vs baseline: 1.0234x; 1.0234x over previous
"""Trainium2 Bass kernel for the block-diagonal equivariant linear
(irreps 256x0e + 256x1o + 128x2e, B=32768, D=1664) on 8 NeuronCores.

Strategy: data-parallel over batch (4096 rows/core), weights replicated.
All HBM traffic is bf16 and all data-layout work is hoisted to the host,
so the device runs a pure block-diagonal GEMM at the bf16 DMA roofline
(13.6MB in + 13.6MB out per core):

- host pre-casts x to bf16 and pre-arranges it chunk-major: per batch
  chunk, SBUF partition u holds 13 planes x n batch values contiguously
  -> each load is one flat 2D DMA with multi-KB contiguous runs;
- matmuls run weights-stationary / x-moving: out block [128 w, n b] per
  (segment, component, w-chunk), accumulated over u-chunks in one PSUM
  bank; bias is a rank-1 PE matmul (bias x ones) on the scalar blocks;
- output goes back in block-major [128 w, block*n+b] bf16 layout; the
  host de-interleaves features/batch and upcasts to fp32 per chunk.

Pipeline shaping: weights (shrunk to [128,1152] + a [1,768] bias/ones
row) lead the Sync HWDGE ring, then all loads follow on it; stores ride
the Scalar HWDGE ring and each chunk is stored in two halves (blocks
0-6 / 7-12, separate SBUF tiles) so the store stream starts as early as
possible; the first chunks are small (128/384 rows) to shorten the
pipeline ramp.  PSUM->SBUF casts alternate Vector/Scalar.
"""

import math
import sys

if "/opt/trn_rl_repo" not in sys.path:
    sys.path.insert(0, "/opt/trn_rl_repo")

import ml_dtypes
import numpy as np

import concourse.tile as tile
from concourse import bacc, mybir
from concourse.bass_utils import run_bass_kernel_spmd

# Problem constants (hardcoded; see module docstring).
DIM = 1664
B_TOTAL = 32768
N_CORES = 8
B_CORE = B_TOTAL // N_CORES  # 4096
CHUNK_B = 512
CH_SIZES = [128, 384] + [512] * 7
assert sum(CH_SIZES) == B_CORE
CH_STARTS = [sum(CH_SIZES[:i]) for i in range(len(CH_SIZES))]
N_CHUNKS = len(CH_SIZES)

# (feature_offset, mul, ir_dim) per segment of the flat feature vector.
SEGS = [(0, 256, 1), (256, 256, 3), (1024, 128, 5)]

# Weight packing columns: per segment, per u-chunk, a [128, mul] block.
_wcols = {}
_wcol = 0
for _si, (s, mul, d) in enumerate(SEGS):
    _wcols[_si] = []
    for c in range(mul // 128):
        _wcols[_si].append(_wcol)
        _wcol += mul
WPK_COLS = _wcol  # 1152
# row tensor (row 0 of wt cols [WPK_COLS:]): bias (256) then ones (512)
BIAS_COL = WPK_COLS
ONES_COL = WPK_COLS + 256
WROW = 256 + CHUNK_B  # 768
WTOT = WPK_COLS + WROW  # 1920

# Input planes: plane (si, i, uc) -> index, in si -> i -> uc order.
PLANE_IDX = {}
PLANE_FEAT = []
for _si, (s, mul, d) in enumerate(SEGS):
    for _i in range(d):
        for _uc in range(mul // 128):
            PLANE_IDX[(_si, _i, _uc)] = len(PLANE_FEAT)
            PLANE_FEAT.append((s + _i + d * 128 * _uc, d))
N_PLANES = len(PLANE_FEAT)  # 13
assert N_PLANES == 13

# Output blocks: block g -> (si, i, wc); out[wc*128+p (partition), b].
# Emission (and copy-completion) order is g0..g12; the chunk is stored
# in two halves: A = blocks 0-6, B = blocks 7-12.
BLOCKS = []
for _wc in range(2):
    for _i in range(3):
        BLOCKS.append((1, _i, _wc))
BLOCKS += [(0, 0, 0), (0, 0, 1)]
BLOCKS += [(2, _i, 0) for _i in range(5)]
G_IDX = {blk: g for g, blk in enumerate(BLOCKS)}
N_BLOCKS = len(BLOCKS)  # 13
N_A = 7  # blocks in store-half A
N_B = N_BLOCKS - N_A  # 6

TOT_COLS = N_PLANES * B_CORE  # flat per-partition DRAM cols (in & out)

# Host-side index tables.
FEAT_OF = np.empty((128, N_PLANES), dtype=np.intp)
for _pl, (_off, _step) in enumerate(PLANE_FEAT):
    FEAT_OF[:, _pl] = _off + _step * np.arange(128)
INV_COL = np.empty(DIM, dtype=np.intp)
for _g, (_si, _i, _wc) in enumerate(BLOCKS):
    _s, _mul, _d = SEGS[_si]
    _w = _wc * 128 + np.arange(128)
    INV_COL[_s + _d * _w + _i] = _g * 128 + np.arange(128)


def _host_weights(ws: np.ndarray, bs: np.ndarray):
    """Pack per-segment weights (1/sqrt(mul) folded in) as [128, 1152]
    bf16, plus a [1, 768] bias+ones row."""
    wpk = np.zeros((128, WPK_COLS), dtype=np.float32)
    off = 0
    for si, (s, mul, d) in enumerate(SEGS):
        w = ws[off : off + mul * mul].reshape(mul, mul) * np.float32(
            1.0 / math.sqrt(mul)
        )
        off += mul * mul
        for c, col in enumerate(_wcols[si]):
            wpk[:, col : col + mul] = w[c * 128 : (c + 1) * 128, :]
    wrow = np.empty((1, WROW), dtype=np.float32)
    wrow[0, :256] = bs
    wrow[0, 256:] = 1.0
    return wpk.astype(ml_dtypes.bfloat16), wrow.astype(ml_dtypes.bfloat16)


def _host_planes(x: np.ndarray) -> np.ndarray:
    """x [B_TOTAL, DIM] fp32 -> xh [N_CORES, 128, TOT_COLS] bf16:
    per chunk, partition u holds plane-major contiguous batch runs."""
    X = x.reshape(N_CORES, B_CORE, DIM)
    xh = np.empty((N_CORES, 128, TOT_COLS), dtype=ml_dtypes.bfloat16)
    for b0, n in zip(CH_STARTS, CH_SIZES):
        cb = N_PLANES * b0
        blk = X[:, b0 : b0 + n, :][:, :, FEAT_OF]  # [C, n, u, pl]
        xh[:, :, cb : cb + N_PLANES * n] = (
            blk.transpose(0, 2, 3, 1).reshape(N_CORES, 128, N_PLANES * n)
        )
    return xh


def _host_out(outs) -> np.ndarray:
    """Device outputs [128, TOT_COLS] bf16 per core -> [B_TOTAL, DIM] fp32."""
    dev = np.stack([np.asarray(o) for o in outs])  # [C, p, cols]
    out = np.empty((N_CORES, B_CORE, DIM), dtype=np.float32)
    for b0, n in zip(CH_STARTS, CH_SIZES):
        cb = N_BLOCKS * b0
        blk = dev[:, :, cb : cb + N_BLOCKS * n].reshape(
            N_CORES, 128, N_BLOCKS, n
        )
        arr = blk.transpose(0, 3, 2, 1).reshape(N_CORES, n, N_BLOCKS * 128)
        out[:, b0 : b0 + n, :] = arr[:, :, INV_COL]
    return out.reshape(B_TOTAL, DIM)


def build_program(x_bufs: int = 5, out_bufs: int = 6, ps_bufs: int = 4):
    """Build + compile the per-core SPMD program. Returns compiled nc."""
    f32 = mybir.dt.float32
    bf16 = mybir.dt.bfloat16

    nc = bacc.Bacc("TRN2", target_bir_lowering=False, debug=False)
    xh_ap = nc.dram_tensor(
        "xh", [128, TOT_COLS], bf16, kind="ExternalInput"
    ).ap()
    wpk_ap = nc.dram_tensor(
        "wpk", [128, WPK_COLS], bf16, kind="ExternalInput"
    ).ap()
    wrow_ap = nc.dram_tensor("wrow", [1, WROW], bf16, kind="ExternalInput").ap()
    out_ap = nc.dram_tensor(
        "out", [128, TOT_COLS], bf16, kind="ExternalOutput"
    ).ap()

    with tile.TileContext(nc) as tc:
        with (
            tc.tile_pool(name="consts", bufs=1) as cpool,
            tc.tile_pool(name="x", bufs=x_bufs) as x_pool,
            tc.tile_pool(name="outs", bufs=out_bufs) as out_pool,
            tc.tile_pool(name="psO", bufs=ps_bufs, space="PSUM") as psO_pool,
        ):
            # weights lead the Sync ring so they land before load 0 ends
            wt = cpool.tile([128, WTOT], bf16)
            nc.sync.dma_start(wt[:, :WPK_COLS], wpk_ap[:])
            nc.sync.dma_start(wt[0:1, WPK_COLS:], wrow_ap[:])

            x_tiles = {}
            out_tiles = {}

            def load_chunk(ch):
                b0, n = CH_STARTS[ch], CH_SIZES[ch]
                cb = N_PLANES * b0
                xt = x_pool.tile([128, N_PLANES * CHUNK_B], bf16, tag="x")
                nc.sync.dma_start(
                    xt[:, : N_PLANES * n], xh_ap[:, cb : cb + N_PLANES * n]
                )
                x_tiles[ch] = xt
                ov = out_pool.tile(
                    [128, N_BLOCKS * CHUNK_B], bf16, tag="outs", name=f"out{ch}"
                )
                out_tiles[ch] = ov

            def store_chunk(ch):
                b0, n = CH_STARTS[ch], CH_SIZES[ch]
                cb = N_BLOCKS * b0
                # one full-chunk store: its 13KB/partition descriptors match
                # the loads', so the SDMA packet round-robin splits HBM
                # bandwidth evenly between the two streams.  Tail-chunk
                # stores ride the Sync ring, idle once all loads are issued.
                eng = nc.sync if ch >= N_CHUNKS - 3 else nc.scalar
                eng.dma_start(
                    out_ap[:, cb : cb + N_BLOCKS * n],
                    out_tiles.pop(ch)[:, : N_BLOCKS * n],
                )

            # PSUM blocks are paired into [128, 2*CHUNK_B] tiles so each
            # PSUM->SBUF cast moves two blocks per instruction (amortizes
            # the fixed per-instruction engine cost).  Pairs never cross
            # the A/B store-half boundary.
            PAIRS = [(0, 1), (2, 3), (4, 5), (6,), (7, 8), (9, 10), (11, 12)]
            PAIR_OF = {}
            for _pi, _pr in enumerate(PAIRS):
                for _j, _g in enumerate(_pr):
                    PAIR_OF[_g] = (_pi, _j)

            copy_flip = [0]

            def chunk_phase(ch):
                n = CH_SIZES[ch]
                xt = x_tiles.pop(ch)
                ov = out_tiles[ch]
                pairtiles = {}
                pair_done = {}

                def pslice(g):
                    # pair partners sit at CHUNK_B-aligned offsets so a
                    # matmul output region never crosses a PSUM bank
                    pi, j = PAIR_OF[g]
                    if pi not in pairtiles:
                        pairtiles[pi] = psO_pool.tile(
                            [128, 2 * CHUNK_B], f32, tag="psO",
                            name=f"ps{ch}_{pi}",
                        )
                        pair_done[pi] = 0
                    t = pairtiles[pi]
                    return t[:, j * CHUNK_B : j * CHUNK_B + n]

                def ovdst(g, w):
                    return ov[:, g * n : g * n + w]

                def emit1(dst, src):
                    if copy_flip[0] % 2 == 0:
                        nc.vector.tensor_copy(dst, src)
                    else:
                        nc.scalar.copy(dst, src)
                    copy_flip[0] += 1

                def finish(g):
                    """Mark block g complete; copy its pair out when full.
                    Full pairs are copied as two halves issued to BOTH copy
                    engines at once -- the PSUM pair frees ~2x sooner, which
                    is what paces the PE."""
                    pi, _ = PAIR_OF[g]
                    pair_done[pi] += 1
                    if pair_done[pi] < len(PAIRS[pi]):
                        return
                    t = pairtiles[pi]
                    gs = PAIRS[pi]
                    if len(gs) == 1:
                        emit1(ovdst(gs[0], n), t[:, :n])
                    else:
                        for j, gj in enumerate(gs):
                            emit1(
                                ovdst(gj, n),
                                t[:, j * CHUNK_B : j * CHUNK_B + n],
                            )

                def xpl(si, i, uc):
                    pl = PLANE_IDX[(si, i, uc)]
                    return xt[:, pl * n : pl * n + n]

                def wblk(si, uc, wc):
                    c0 = _wcols[si][uc] + wc * 128
                    return wt[:, c0 : c0 + 128]

                # seg1: 3 components x 2 w-chunks, accumulate over u-chunks;
                # i-inner so each stationary weight block feeds 3 matmuls
                for wc in range(2):
                    for uc in range(2):
                        for i in range(3):
                            nc.tensor.matmul(
                                pslice(G_IDX[(1, i, wc)]),
                                wblk(1, uc, wc),
                                xpl(1, i, uc),
                                start=(uc == 0),
                                stop=(uc == 1),
                            )
                    for i in range(3):
                        finish(G_IDX[(1, i, wc)])
                # seg0: 2 w-chunks, accumulate over u-chunks + rank-1 bias
                for wc in range(2):
                    g = G_IDX[(0, 0, wc)]
                    for uc in range(2):
                        nc.tensor.matmul(
                            pslice(g),
                            wblk(0, uc, wc),
                            xpl(0, 0, uc),
                            start=(uc == 0),
                            stop=False,
                        )
                    nc.tensor.matmul(
                        pslice(g),
                        wt[0:1, BIAS_COL + wc * 128 : BIAS_COL + wc * 128 + 128],
                        wt[0:1, ONES_COL : ONES_COL + n],
                        start=False,
                        stop=True,
                    )
                    finish(g)
                # seg2: 5 components, single u-chunk (shared stationary)
                for i in range(5):
                    g = G_IDX[(2, i, 0)]
                    nc.tensor.matmul(
                        pslice(g), wblk(2, 0, 0), xpl(2, i, 0),
                        start=True, stop=True,
                    )
                    finish(g)
                store_chunk(ch)

            for ch in range(min(x_bufs, N_CHUNKS)):
                load_chunk(ch)
            loaded = min(x_bufs, N_CHUNKS)
            for ch in range(N_CHUNKS):
                if loaded < N_CHUNKS:
                    load_chunk(loaded)
                    loaded += 1
                chunk_phase(ch)

    nc.compile()
    return nc


_CACHE: dict = {}


def prep_in_maps(ws: np.ndarray, bs: np.ndarray, x: np.ndarray):
    wpk, wrow = _host_weights(
        np.asarray(ws, dtype=np.float32), np.asarray(bs, dtype=np.float32)
    )
    xh = _host_planes(np.asarray(x, dtype=np.float32))
    return [{"xh": xh[i], "wpk": wpk, "wrow": wrow} for i in range(N_CORES)]


def kernel(ws: np.ndarray, bs: np.ndarray, x: np.ndarray) -> np.ndarray:
    if "nc" not in _CACHE:
        _CACHE["nc"] = build_program()
    nc = _CACHE["nc"]
    in_maps = prep_in_maps(ws, bs, x)
    res = run_bass_kernel_spmd(nc, in_maps, list(range(N_CORES)))
    return _host_out([r["out"] for r in res.results])
